# revision 16
# baseline (speedup 1.0000x reference)
"""Multi-head attention (B=2, S=2048, D=1024, H=16) on 8 TRN2 NeuronCores.

Sharding: batch (2) x head-groups (4 heads/core). Each core computes its
batch's QKV projections for its 4 heads, causal attention, and a partial
output projection over its head slice; the host sums the 4 partials per
batch and adds the output bias.

Layout strategy: everything runs in "transposed" orientation so no on-chip
transposes are needed:
  q2^T[dm, s] = Wq[dm,:] @ Q^T       (host supplies Q^T and Wq^T)
  scores^T[j, si] = k^T.T @ q^T      (d_h contraction, 2 heads row-tiled)
  attn^T = exp(scores^T/8 + mask)    (no row-max: |scores| < ~4)
  ctx^T+denom = [v | 1].T @ attn^T   (ones column gives softmax denominator)
  out[s, n] = ctxn^T.T @ Wo^T        (K=65 chunks; Wo row 64 zero-padded)
Matmuls use float32r (full PE rate at free-dim>=256, ~1.6e-4 rel err).
"""

import numpy as np

B, S, D, H, DH = 2, 2048, 1024, 16, 64
NCORES = 8
CORES_PER_BATCH = 4
HPC = H // CORES_PER_BATCH  # heads per core = 4
NEG = -60000.0  # exp(NEG/8) == 0 exactly in fp32; fits in fp16
MMDT = "f16"  # "f32r" | "f16" | "bf16"

TRACE = False  # test.py sets True to collect an NTFF profile
LAST_RESULT = None  # BassKernelResults from the last run (for test.py)

_built = {}


def _build(causal: bool, mmdt: str):
    key = (causal, mmdt)
    if key in _built:
        return _built[key]
    import concourse.mybir as mybir
    import concourse.tile as tile
    from concourse import bacc
    from concourse.bass import ts, ds

    f32 = mybir.dt.float32
    f32r = mybir.dt.float32r
    DT = {"f32r": mybir.dt.float32r, "f16": mybir.dt.float16,
          "bf16": mybir.dt.bfloat16}[mmdt]
    DTNP = {"f32r": f32, "f16": mybir.dt.float16, "bf16": mybir.dt.bfloat16}[mmdt]
    EXP = mybir.ActivationFunctionType.Exp

    nc = bacc.Bacc("TRN2")
    qt = nc.dram_tensor("qt", [D, S], DTNP, kind="ExternalInput")
    kt = nc.dram_tensor("kt", [D, S], DTNP, kind="ExternalInput")
    vt = nc.dram_tensor("vt", [D, S], DTNP, kind="ExternalInput")
    wq = nc.dram_tensor("wq", [D, HPC * DH], DTNP, kind="ExternalInput")
    wk = nc.dram_tensor("wk", [D, HPC * DH], DTNP, kind="ExternalInput")
    wv = nc.dram_tensor("wv", [D, HPC * DH], DTNP, kind="ExternalInput")
    wo = nc.dram_tensor("wo", [HPC, DH + 1, D], DTNP, kind="ExternalInput")
    bq = nc.dram_tensor("bq", [128, 2], f32, kind="ExternalInput")
    bk = nc.dram_tensor("bk", [128, 2], f32, kind="ExternalInput")
    bv = nc.dram_tensor("bv", [1, HPC * DH], DTNP, kind="ExternalInput")
    ident = nc.dram_tensor("ident", [128, 128], DTNP, kind="ExternalInput")
    if causal:
        mp = nc.dram_tensor("mp", [128, 4, 512], DTNP, kind="ExternalInput")
    else:
        mt = nc.dram_tensor("mt", [S, S], DTNP, kind="ExternalInput")
    out = nc.dram_tensor("out", [S, D], f32, kind="ExternalOutput")

    NSB = S // 512   # 4 si-blocks of 512
    NST = S // 128   # 16 s-tiles / j-tiles of 128

    import contextlib
    with tile.TileContext(nc) as tc, contextlib.ExitStack() as ctx_pools:
        with (
            tc.tile_pool(name="persist", bufs=1) as pp,
            tc.tile_pool(name="sc_ps", bufs=2, space="PSUM") as sc_ps,
            tc.tile_pool(name="ctx_ps", bufs=1, space="PSUM") as ctx_ps,
            tc.tile_pool(name="mm_ps", bufs=2, space="PSUM") as mm_ps,
        ):
            # ---- persistent tiles ----
            bq_t = pp.tile([128, 2], f32)
            nc.sync.dma_start(out=bq_t, in_=bq[:, :])
            bk_t = pp.tile([128, 2], f32)
            nc.sync.dma_start(out=bk_t, in_=bk[:, :])
            bv_t = pp.tile([1, HPC * DH], DT)
            nc.sync.dma_start(out=bv_t, in_=bv[:, :].bitcast(DT))
            if causal:
                mp_t = pp.tile([128, 4, 512], DT)
                nc.sync.dma_start(out=mp_t, in_=mp[:, :, :].bitcast(DT))
            id_t = pp.tile([128, 128], DT)
            nc.sync.dma_start(out=id_t, in_=ident[:, :].bitcast(DT))
            ones_c = pp.tile([1, 128], DT)
            nc.vector.memset(ones_c.bitcast(DTNP), 1.0)
            ones_b = pp.tile([1, 64], f32r)
            nc.vector.memset(ones_b.bitcast(f32), 1.0)

            ap = ctx_pools.enter_context(tc.tile_pool(name="attn", bufs=4))
            smp = ctx_pools.enter_context(tc.tile_pool(name="small", bufs=3))
            mlp = None if causal else ctx_pools.enter_context(tc.tile_pool(name="mload", bufs=3))
            owp = ctx_pools.enter_context(tc.tile_pool(name="outw", bufs=1))
            op = ctx_pools.enter_context(tc.tile_pool(name="outp", bufs=3))

            q2t = [pp.tile([128, S], DT, tag=f"q2t{i}", name=f"q2t{i}") for i in range(2)]
            k2t = [pp.tile([128, S], DT, tag=f"k2t{i}", name=f"k2t{i}") for i in range(2)]
            vaug = pp.tile([128, NST, HPC * (DH + 1)], DT)
            for h in range(HPC):
                nc.vector.memset(vaug[:, :, ds(h * 65 + 64, 1)].bitcast(DTNP), 1.0)
            ctxt = [pp.tile([DH + 1, S], DT, tag=f"ctxt{h}", name=f"ctxt{h}") for h in range(HPC)]

            # ---- phase 1: QKV projections ----
            with (
                tc.tile_pool(name="wproj", bufs=1) as wp,
                tc.tile_pool(name="stream", bufs=2) as sp,
            ):
                wq_t = wp.tile([128, 8, 256], DT)
                nc.sync.dma_start(out=wq_t, in_=wq.rearrange("(c p) m -> p c m", p=128).bitcast(DT))
                wk_t = wp.tile([128, 8, 256], DT)
                nc.sync.dma_start(out=wk_t, in_=wk.rearrange("(c p) m -> p c m", p=128).bitcast(DT))
                wv_t = wp.tile([128, 8, 256], DT)
                nc.sync.dma_start(out=wv_t, in_=wv.rearrange("(c p) m -> p c m", p=128).bitcast(DT))

                for sb in range(NSB):
                    qs = [sp.tile([128, 4, 512], DT, tag=f"qs{i}", name=f"qs{i}", bufs=2) for i in range(2)]
                    ks = [sp.tile([128, 4, 512], DT, tag=f"ks{i}", name=f"ks{i}", bufs=2) for i in range(2)]
                    vs = [sp.tile([128, 4, 512], DT, tag=f"vs{i}", name=f"vs{i}", bufs=2) for i in range(2)]
                    for half in range(2):
                        for name, t, dr in (("q", qs, qt), ("k", ks, kt), ("v", vs, vt)):
                            src = dr.rearrange("(c p) s -> p c s", p=128)
                            nc.sync.dma_start(
                                out=t[half],
                                in_=src[:, ds(half * 4, 4), ts(sb, 512)].bitcast(DT),
                            )
                    for hp in range(2):
                        psq = mm_ps.tile([128, 512], f32, tag="mm")
                        for c in range(8):
                            nc.tensor.matmul(
                                psq, wq_t[:, c, ts(hp, 128)], qs[c // 4][:, c % 4, :],
                                start=(c == 0), stop=(c == 7),
                            )
                        nc.vector.tensor_scalar_add(
                            q2t[hp][:, ts(sb, 512)], psq, bq_t[:, ds(hp, 1)])
                        psk = mm_ps.tile([128, 512], f32, tag="mm")
                        for c in range(8):
                            nc.tensor.matmul(
                                psk, wk_t[:, c, ts(hp, 128)], ks[c // 4][:, c % 4, :],
                                start=(c == 0), stop=(c == 7),
                            )
                        nc.vector.tensor_scalar_add(
                            k2t[hp][:, ts(sb, 512)], psk, bk_t[:, ds(hp, 1)])
                    for st4 in range(4):
                        st = sb * 4 + st4
                        psv = mm_ps.tile([128, 256], f32, tag="mm")
                        for c in range(8):
                            nc.tensor.matmul(
                                psv, vs[c // 4][:, c % 4, ts(st4, 128)], wv_t[:, c, :],
                                start=(c == 0), stop=False,
                            )
                        nc.tensor.matmul(psv, ones_c, bv_t, start=False, stop=True)
                        nc.vector.tensor_copy(
                            vaug[:, st, :].rearrange("p (h x) -> p h x", h=HPC)[:, :, 0:DH],
                            psv.rearrange("p (h x) -> p h x", h=HPC),
                        )

            # ---- phase 2: attention ----
            if True:
                for hp in range(2):
                    for sb in range(NSB):
                        jts = list(range(4 * sb + 4)) if causal else list(range(NST))
                        cps = [ctx_ps.tile([DH + 1, 512], f32, tag=f"ctx{a}", name=f"cps{a}")
                               for a in range(2)]
                        for jt in jts:
                            sc = sc_ps.tile([128, 1024], f32, tag="sc")
                            straddle = causal and jt >= 4 * sb
                            mt_t = None
                            if not causal:
                                mt_t = mlp.tile([128, 512], DT, tag="mt")
                                nc.sync.dma_start(
                                    out=mt_t,
                                    in_=mt[ts(jt, 128), ts(sb, 512)].bitcast(DT))
                            for a in range(2):
                                masked_tile = straddle or not causal
                                nc.tensor.matmul(
                                    sc[:, ts(a, 512)],
                                    k2t[hp][ds(a * 64, 64), ts(jt, 128)],
                                    q2t[hp][ds(a * 64, 64), ts(sb, 512)],
                                    start=True, stop=not masked_tile,
                                    tile_position=(a * 64, 0),
                                )
                                if masked_tile:
                                    nc.tensor.matmul(
                                        sc[:, ts(a, 512)], id_t,
                                        mp_t[:, jt - 4 * sb, :] if causal else mt_t,
                                        start=False, stop=True,
                                    )
                            at = ap.tile([128, 1024], DT, tag="at")
                            nc.scalar.activation(at, sc, EXP, scale=0.125)
                            for a in range(2):
                                h = 2 * hp + a
                                nc.tensor.matmul(
                                    cps[a],
                                    vaug[:, jt, ds(h * 65, DH + 1)],
                                    at[:, ts(a, 512)],
                                    start=(jt == jts[0]), stop=(jt == jts[-1]),
                                )
                        for a in range(2):
                            h = 2 * hp + a
                            cu = smp.tile([DH + 1, 512], f32, tag="norm", name="cu")
                            nc.vector.tensor_copy(cu, cps[a])
                            rd = smp.tile([1, 512], f32r, tag="rd", name="rd")
                            with nc.allow_low_precision(reason="f32r feeds PE bcast"):
                                nc.vector.reciprocal(rd, cu[ds(DH, 1), :])
                            bc = mm_ps.tile([64, 512], f32, tag="mm")
                            nc.tensor.matmul(bc, ones_b, rd, start=True, stop=True)
                            nc.vector.tensor_mul(ctxt[h][0:DH, ts(sb, 512)], cu[0:DH, :], bc)
                for h in range(HPC):
                    nc.vector.memset(ctxt[h][ds(DH, 1), :].bitcast(DTNP), 0.0)

            # ---- phase 3: output projection (partial over this core's heads) ----
            if True:
                wo_t = owp.tile([DH + 1, HPC, D], DT)
                nc.sync.dma_start(out=wo_t, in_=wo.rearrange("h p n -> p h n").bitcast(DT))
                for st in range(NST):
                    for nb in range(2):
                        po = mm_ps.tile([128, 512], f32, tag="mm")
                        for h in range(HPC):
                            nc.tensor.matmul(
                                po, ctxt[h][:, ts(st, 128)], wo_t[:, h, ts(nb, 512)],
                                start=(h == 0), stop=(h == HPC - 1),
                            )
                        ot = op.tile([128, 512], f32, tag="ot")
                        nc.vector.tensor_copy(ot, po)
                        nc.sync.dma_start(out=out[ts(st, 128), ts(nb, 512)], in_=ot)

            ctx_pools.close()

    nc.finalize()
    _built[key] = nc
    return nc


def _null_ctx():
    import contextlib
    return contextlib.nullcontext()


def _is_causal(masked: np.ndarray) -> bool:
    c = ~np.tril(np.ones((S, S), dtype=bool))
    return all(np.array_equal(masked[b], c) for b in range(masked.shape[0]))


def kernel(Q, K, V, masked, WQ_w, WQ_b, WK_w, WK_b, WV_w, WV_b, Wo_w, Wo_b):
    global LAST_RESULT
    from concourse.bass_utils import run_bass_kernel_spmd

    Q = np.asarray(Q, dtype=np.float32)
    K = np.asarray(K, dtype=np.float32)
    V = np.asarray(V, dtype=np.float32)
    masked = np.asarray(masked)
    causal = _is_causal(masked)
    nc = _build(causal, MMDT)
    if MMDT == "f16":
        npdt = np.float16
    elif MMDT == "bf16":
        import ml_dtypes
        npdt = ml_dtypes.bfloat16
    else:
        npdt = np.float32

    qT = [np.ascontiguousarray(Q[b].T.astype(npdt)) for b in range(B)]
    kT = [np.ascontiguousarray(K[b].T.astype(npdt)) for b in range(B)]
    vT = [np.ascontiguousarray(V[b].T.astype(npdt)) for b in range(B)]

    if causal:
        # mp[p, d, f'] (f' = a*512 + f, duplicated halves): -1e9 where (d*128+p) > f
        p = np.arange(128)[:, None, None]
        d = np.arange(4)[None, :, None]
        f = np.arange(512)[None, None, :]
        mp_full = np.ascontiguousarray(
            np.where(d * 128 + p > f, np.float32(NEG), np.float32(0.0)).astype(npdt))
    else:
        mtb = [np.ascontiguousarray(
            np.where(masked[b].T, np.float32(NEG), np.float32(0.0)).astype(npdt))
            for b in range(B)]

    in_maps = []
    for c in range(NCORES):
        b = c // CORES_PER_BATCH
        h0 = (c % CORES_PER_BATCH) * HPC
        sel = slice(h0 * DH, (h0 + HPC) * DH)
        wo_pad = np.zeros((HPC, DH + 1, D), np.float32)
        wo_pad[:, :DH, :] = Wo_w.T[sel].reshape(HPC, DH, D)
        m = {
            "qt": qT[b], "kt": kT[b], "vt": vT[b],
            "wq": np.ascontiguousarray(np.asarray(WQ_w)[sel].T.astype(npdt)),
            "wk": np.ascontiguousarray(np.asarray(WK_w)[sel].T.astype(npdt)),
            "wv": np.ascontiguousarray(np.asarray(WV_w)[sel].T.astype(npdt)),
            "wo": wo_pad.astype(npdt),
            "bq": np.ascontiguousarray(np.asarray(WQ_b)[sel].reshape(2, 128).T),
            "bk": np.ascontiguousarray(np.asarray(WK_b)[sel].reshape(2, 128).T),
            "bv": np.ascontiguousarray(np.asarray(WV_b)[sel].reshape(1, HPC * DH).astype(npdt)),
            "ident": np.eye(128).astype(npdt),
        }
        if causal:
            m["mp"] = mp_full
        else:
            m["mt"] = mtb[b]
        m["bq"] = m["bq"].astype(np.float32)
        m["bk"] = m["bk"].astype(np.float32)
        m = {k: np.ascontiguousarray(v) for k, v in m.items()}
        in_maps.append(m)

    res = run_bass_kernel_spmd(nc, in_maps, core_ids=list(range(NCORES)), trace=TRACE)
    LAST_RESULT = res

    acc = np.zeros((B, S, D), np.float64)
    for c in range(NCORES):
        acc[c // CORES_PER_BATCH] += res.results[c]["out"].astype(np.float64)
    acc += np.asarray(Wo_b, dtype=np.float64)[None, None, :]
    return acc.astype(np.float32)
